# revision 1
# baseline (speedup 1.0000x reference)
"""Trainium2 Bass kernel for nn_MergeZoom: per-sample mask bbox + crop + bilinear resize.

Algorithm (per sample, all on-device):
  mb   = (mask >= 0.5)
  rows/cols nonzero -> bbox (first,last per axis) via exact count/weighted-sum trick
  out  = R @ (mb * image) @ C^T  where R/C are bilinear "tent" matrices built on-chip
         (stored negated: -relu(1 - |src - h|); the negations cancel across stages).
  stage1: T1t[w, ho] = (Mh)^T_as_lhsT . RT   (contracts h)
  stage2: out[ho,wo] = (T1t)^T_as_lhsT . CT  (contracts w)

Layout: rows interleaved h = 4p + t (p = SBUF partition, t = chunk) so every DMA is
one fully contiguous block per partition. Stage-1 output columns are permuted
(sigma(j) = 4*(j%128) + j//128) so stage-2 slices produce the same interleaved rows.
Output is written bf16 (per-row-chunk DMAs fired as soon as each chunk's PSUM is
evacuated) and upcast on host.

Emission is software-pipelined two-deep: prep_early(s+2) [mask DMA, binarize,
row-counts, colsum matmuls] is emitted before compute(s) so each engine's in-order
queue always has the next samples' prerequisites ahead of the current sample's
PSUM evacuations; the PE never waits on masks/tents.

Sharding: pure data-parallel, 4 samples per core across 8 cores.
"""

import numpy as np

import concourse.bass as bass
import concourse.tile as tile
from concourse import bacc, mybir

B = 32
N_CORES = 8
BPC = B // N_CORES  # samples per core
H = W = 512
C = 3
HT = H // 128  # 4 h-chunks of 128 partitions
WT = W // 128

FP = mybir.dt.float32
BF = mybir.dt.bfloat16
AX = mybir.AxisListType.X
OP = mybir.AluOpType
AF = mybir.ActivationFunctionType


def build(bpc: int = BPC) -> bass.Bass:
    nc = bacc.Bacc()
    mask_d = nc.declare_dram_parameter("mask", [bpc, H, W, 1], FP, isOutput=False)
    img_d = nc.declare_dram_parameter("image", [bpc, H, W, C], FP, isOutput=False)
    constf_d = nc.declare_dram_parameter(
        "constf", [128, 1024 + 4 * HT], FP, isOutput=False
    )
    out_d = nc.declare_dram_parameter("out", [bpc, C, H, W], BF, isOutput=True)

    with tile.TileContext(nc) as tc:
        with (
            tc.tile_pool(name="consts", bufs=1) as cpool,
            tc.tile_pool(name="io", bufs=2) as iopool,
            tc.tile_pool(name="work", bufs=2) as wk,
            tc.tile_pool(name="tents", bufs=2) as wkt,
            tc.tile_pool(name="small", bufs=2) as sm,
            tc.tile_pool(name="ps1", bufs=3, space="PSUM") as ps1p,
            tc.tile_pool(name="ps2", bufs=3, space="PSUM") as ps2p,
            tc.tile_pool(name="psx", bufs=2, space="PSUM") as psxp,
        ):
            # consts triggered from the scalar queue; the sync queue starts on
            # the first mask immediately
            constf = cpool.tile([128, 1024 + 4 * HT], FP)
            nc.scalar.dma_start(constf[:], constf_d[:])
            iota = constf[:, 0:512]  # 0..511 (cols)
            iotap = constf[:, 512:1024]  # sigma(j) permuted (rows)
            negp = constf[:, 1024 : 1024 + 2 * HT]  # [-(4p+t) | -(128w+p)]
            tp = constf[:, 1024 + 2 * HT : 1024 + 4 * HT]  # [t | p] fp32
            onesh = cpool.tile([128, 128], BF)
            nc.vector.memset(onesh[:], 1.0)
            onesf = cpool.tile([128, 128], FP)
            nc.vector.memset(onesf[:], 1.0)

            state: dict[int, dict] = {}

            def prep_early(s: int):
                st: dict = {}
                state[s] = st
                # mask first; image in 4 row-chunks so masking can start early.
                # sample 0 is latency-critical: chunk its mask DMA + binarize so
                # colsums/row-counts/gps-multiplies start as data streams in.
                msk = iopool.tile([128, HT * 512], FP, tag="msk")
                msrc = mask_d[s].rearrange("(p t) w one -> p t (w one)", t=HT)
                nmc = 1
                for t in range(nmc):
                    lo_, hi_ = t * (HT // nmc), (t + 1) * (HT // nmc)
                    nc.sync.dma_start(
                        msk[:, lo_ * 512 : hi_ * 512], msrc[:, lo_:hi_, :]
                    )
                img = iopool.tile([128, HT * 512 * C], FP, tag="img")
                imgsrc = img_d[s].rearrange("(p t) w c -> p t (w c)", t=HT)
                # chunked only for sample 0 (masking there is latency-critical);
                # later samples have slack, and fewer triggers free the sync queue
                nic = HT
                for t in range(nic):
                    lo_, hi_ = t * (HT // nic), (t + 1) * (HT // nic)
                    nc.sync.dma_start(
                        img[:, lo_ * 1536 : hi_ * 1536],
                        imgsrc[:, lo_:hi_, :],
                    )
                st["img"] = img

                mbh = wk.tile([128, HT * 512], BF, tag="mbh")
                pscols = psxp.tile([128, 512], FP, tag="pscols")
                for t in range(nmc):
                    lo_, hi_ = t * (HT // nmc), (t + 1) * (HT // nmc)
                    nc.vector.tensor_scalar(
                        mbh[:, lo_ * 512 : hi_ * 512],
                        msk[:, lo_ * 512 : hi_ * 512],
                        0.5, None, OP.is_ge,
                    )
                    for u in range(lo_, hi_):
                        nc.tensor.matmul(
                            pscols[:],
                            onesh[:],
                            mbh[:, u * 512 : (u + 1) * 512],
                            start=(u == 0),
                            stop=(u == HT - 1),
                        )
                st["mbh"] = mbh
                st["pscols"] = pscols

            def prep_late(s: int):
                """Generator: yields at op boundaries so compute() can weave
                these ops between its PSUM evacuations in each engine queue."""
                st = state[s]
                img, mbh, pscols = st["img"], st["mbh"], st["pscols"]

                # row counts on the scalar engine (accum_out = per-partition sum);
                # emitted here so they sit behind the previous sample's evacuations
                r4 = sm.tile([128, HT], FP, tag="r4")
                junk = sm.tile([128, HT * 512], BF, tag="junk")
                for t in range(HT):
                    nc.scalar.activation(
                        junk[:, t * 512 : (t + 1) * 512],
                        mbh[:, t * 512 : (t + 1) * 512],
                        AF.Copy,
                        accum_out=r4[:, t : t + 1],
                    )
                    if t % 2 == 1:
                        yield

                colnz = sm.tile([128, 512], BF, tag="colnz")
                nc.vector.tensor_scalar(colnz[:], pscols[:], 0.0, None, OP.is_gt)
                wcol = sm.tile([128, 512], FP, tag="wcol")
                nc.vector.tensor_tensor(wcol[:], iota, colnz[:], OP.mult)
                yield

                # NS = [Nr, Nc, Sr, Sc]; col sums via scalar-engine accum
                NS = sm.tile([128, 4], FP, tag="NS")
                jc = sm.tile([128, 1024], FP, tag="jc")
                nc.scalar.activation(
                    jc[:, 0:512], colnz[:], AF.Copy, accum_out=NS[:, 1:2]
                )
                nc.scalar.activation(
                    jc[:, 512:1024], wcol[:], AF.Copy, accum_out=NS[:, 3:4]
                )
                yield

                # rows: rwh = [nz | nz*t | nz*p] fp32 (tiny bf16 DVE ops are ~10x
                # slower than fp32); partition sums via fp32 ones-matmul
                rwh = sm.tile([128, 3 * HT], FP, tag="rwh")
                nc.vector.tensor_scalar(rwh[:, 0:HT], r4[:], 0.0, None, OP.is_gt)
                nc.vector.tensor_tensor(
                    rwh[:, HT : 2 * HT], rwh[:, 0:HT], tp[:, 0:HT], OP.mult
                )
                nc.vector.tensor_tensor(
                    rwh[:, 2 * HT : 3 * HT], rwh[:, 0:HT], tp[:, HT : 2 * HT], OP.mult
                )
                nc.tensor.matmul(
                    pscols[:, 0 : 3 * HT], onesf[:], rwh[:], start=True, stop=True
                )
                yield
                r3 = sm.tile([128, 3], FP, tag="r3")
                nc.vector.tensor_reduce(
                    r3[:], pscols[:, 0 : 3 * HT].rearrange("p (i t) -> p i t", i=3),
                    AX, OP.add,
                )
                nc.vector.tensor_copy(NS[:, 0:1], r3[:, 0:1])
                # h = 4p + t -> Sr = 4*Sr_p + Sr_t
                sr4 = sm.tile([128, 1], FP, tag="sr4")
                nc.vector.tensor_scalar(sr4[:], r3[:, 2:3], 4.0, None, OP.mult)
                nc.vector.tensor_tensor(NS[:, 2:3], sr4[:], r3[:, 1:2], OP.add)
                yield

                # ---------------- bbox scalars ([128,2]: rows, cols) ----------------
                ch = sm.tile([128, 20], FP, tag="ch")
                N2, S2 = NS[:, 0:2], NS[:, 2:4]
                rec, mean = ch[:, 0:2], ch[:, 2:4]
                half, first = ch[:, 4:6], ch[:, 6:8]
                av, qv = ch[:, 8:10], ch[:, 10:12]
                fm, bv = ch[:, 12:14], ch[:, 14:16]
                lo, hi = ch[:, 16:18], ch[:, 18:20]
                nc.vector.reciprocal(rec, N2)
                nc.vector.tensor_tensor(mean, S2, rec, OP.mult)
                nc.vector.tensor_scalar(half, N2, 0.5, 0.5, OP.mult, OP.subtract)
                nc.vector.tensor_tensor(first, mean, half, OP.subtract)
                yield
                nc.vector.tensor_scalar(av, N2, 1.0, 1.0 / 512.0, OP.add, OP.mult)
                nc.vector.tensor_scalar(qv, N2, 1.0, 1.0 / 1024.0, OP.add, OP.mult)
                nc.vector.tensor_scalar(fm, first, 1.5, None, OP.subtract)
                nc.vector.tensor_tensor(bv, fm, qv, OP.add)
                nc.vector.tensor_scalar(lo, first, 1.0, None, OP.subtract)
                nm1 = sm.tile([128, 2], FP, tag="nm1")
                nc.vector.tensor_scalar(nm1[:], N2, 1.0, None, OP.subtract)
                nc.vector.tensor_tensor(hi, nm1[:], first, OP.add)
                yield

                # ---------------- src vectors + negated tent matrices ---------------
                RT = wkt.tile([128, HT * 512], BF, tag="RT")
                CT = wkt.tile([128, WT * 512], BF, tag="CT")
                st["RT"], st["CT"] = RT, CT
                srcR = sm.tile([128, 512], FP, tag="srcR")
                srcC = sm.tile([128, 512], FP, tag="srcC")
                nc.vector.tensor_scalar(
                    srcR[:], iotap, av[:, 0:1], bv[:, 0:1], OP.mult, OP.add
                )
                nc.vector.tensor_scalar(
                    srcR[:], srcR[:], lo[:, 0:1], hi[:, 0:1], OP.max, OP.min
                )
                yield
                nc.vector.tensor_scalar(
                    srcC[:], iota, av[:, 1:2], bv[:, 1:2], OP.mult, OP.add
                )
                nc.vector.tensor_scalar(
                    srcC[:], srcC[:], lo[:, 1:2], hi[:, 1:2], OP.max, OP.min
                )
                yield
                for o, src, mat in ((0, srcR, RT), (HT, srcC, CT)):
                    for t in range(HT):
                        e = sm.tile([128, 512], FP, tag="e")
                        nc.scalar.activation(
                            e[:], src[:], AF.Abs, bias=negp[:, o + t : o + t + 1],
                            scale=1.0,
                        )
                        nc.vector.tensor_scalar(
                            mat[:, t * 512 : (t + 1) * 512], e[:], 1.0, 0.0,
                            OP.subtract, OP.min,
                        )
                        yield

                # masked image (bf16) via 0-stride broadcast of mb
                Mh = wk.tile([128, HT * 512 * C], BF, tag="Mh")
                img4 = img[:].rearrange("p (t w c) -> p t w c", t=HT, w=512)
                Mh4 = Mh[:].rearrange("p (t w c) -> p t w c", t=HT, w=512)
                st["Mh4"] = Mh4
                mb4 = (
                    mbh[:]
                    .rearrange("p (t w) -> p t w", t=HT)
                    .unsqueeze(3)
                    .broadcast_to([128, HT, 512, C])
                )
                ngps = HT
                for t in range(ngps):
                    nc.gpsimd.tensor_tensor(Mh4[:, t], img4[:, t], mb4[:, t], OP.mult)
                    yield
                for t in range(ngps, HT):
                    nc.vector.tensor_tensor(Mh4[:, t], img4[:, t], mb4[:, t], OP.mult)
                    yield

            def compute(s: int, g=None):
                def step():
                    pass

                def evac_eng(cp):
                    return nc.vector if cp % 2 == 0 else nc.scalar

                st = state[s]
                Mh4, RT, CT = st["Mh4"], st["RT"], st["CT"]
                # ------ stage 1: T1t[w, ho-permuted] per channel, paired PSUM ------
                t1 = wk.tile([128, C * WT * 512], BF, tag="t1")
                cp = 0
                for c in range(C):
                    for wt in range(WT):
                        ps1 = ps1p.tile([128, 512], FP, tag="ps1")
                        for ht in range(HT):
                            nc.tensor.matmul(
                                ps1[:],
                                Mh4[:, ht, wt * 128 : (wt + 1) * 128, c],
                                RT[:, ht * 512 : (ht + 1) * 512],
                                start=(ht == 0),
                                stop=(ht == HT - 1),
                            )
                        dst = t1[:, (c * WT + wt) * 512 : (c * WT + wt + 1) * 512]
                        eng = evac_eng(cp)
                        if eng is nc.vector:
                            nc.vector.tensor_copy(dst, ps1[:])
                        else:
                            nc.scalar.copy(dst, ps1[:])
                        cp += 1
                        step()

                # -------- stage 2 (ot-major) + per-chunk output DMA --------
                outt = iopool.tile([128, HT * 512 * C], BF, tag="outt")
                out4 = outt[:].rearrange("p (t c w) -> p t c w", t=HT, c=C)
                odst = out_d[s].rearrange("c (p t) w -> p t c w", t=HT)
                for ot in range(HT):
                    for c in range(C):
                        ps2 = ps2p.tile([128, 512], FP, tag="ps2")
                        for wt in range(WT):
                            lhsT2 = t1[
                                :,
                                (c * WT + wt) * 512 + ot * 128 : (c * WT + wt) * 512
                                + (ot + 1) * 128,
                            ]
                            nc.tensor.matmul(
                                ps2[:],
                                lhsT2,
                                CT[:, wt * 512 : (wt + 1) * 512],
                                start=(wt == 0),
                                stop=(wt == WT - 1),
                            )
                        dst = out4[:, ot, c, :]
                        eng = evac_eng(cp)
                        if eng is nc.vector:
                            nc.vector.tensor_copy(dst, ps2[:])
                        else:
                            nc.scalar.copy(dst, ps2[:])
                        cp += 1
                        step()
                    # fire this row-chunk's store as soon as its 3 evacs land.
                    # last chunk triggers from scalar: its own HW-DGE queue, so
                    # the final store overlaps the SWDGE queue's drain (tail)
                    if ot == HT - 1:
                        trig = nc.scalar
                    elif s == bpc - 1:
                        trig = nc.sync  # idle at the end; HW-DGE beats SWDGE
                    else:
                        trig = nc.gpsimd
                    trig.dma_start(
                        odst[:, ot],
                        outt[:, ot * 1536 : (ot + 1) * 1536].rearrange(
                            "p (c w) -> p c w", c=C
                        ),
                    )

            # two-deep software pipeline; prep_late(s+2) is woven between
            # compute(s)'s PSUM evacuations to avoid head-of-line blocking
            prep_early(0)
            for _ in prep_late(0):
                pass
            prep_early(1)
            for _ in prep_late(1):
                pass
            for s in range(bpc):
                if s + 2 < bpc:
                    prep_early(s + 2)
                compute(s)
                if s + 2 < bpc:
                    for _ in prep_late(s + 2):
                        pass
                state.pop(s)

    nc.compile()
    return nc


def make_consts() -> dict[str, np.ndarray]:
    iota_f = np.broadcast_to(np.arange(512, dtype=np.float32), (128, 512))
    j = np.arange(512)
    sigma = 4 * (j % 128) + j // 128
    iota_p = np.broadcast_to(sigma.astype(np.float32), (128, 512))
    p = np.arange(128, dtype=np.float32)
    negr = np.stack([-(4.0 * p + t) for t in range(HT)], axis=1)
    negc = np.stack([-(128.0 * wt + p) for wt in range(WT)], axis=1)
    tvals = np.broadcast_to(np.arange(HT, dtype=np.float32)[None, :], (128, HT))
    pvals = np.broadcast_to(p[:, None], (128, HT))
    constf = np.concatenate(
        [iota_f, iota_p, negr, negc, tvals, pvals], axis=1
    ).astype(np.float32)
    return {"constf": constf}


_NC_CACHE: dict[int, bass.Bass] = {}


def _get_nc(bpc: int = BPC) -> bass.Bass:
    if bpc not in _NC_CACHE:
        _NC_CACHE[bpc] = build(bpc)
    return _NC_CACHE[bpc]


def run(mask: np.ndarray, image: np.ndarray, trace: bool = False, **kwargs):
    """Run on 8 cores; returns (out [B,H,W,C] fp32, BassKernelResults)."""
    from concourse.bass_utils import run_bass_kernel_spmd

    nc = _get_nc(BPC)
    consts = make_consts()
    mask = np.ascontiguousarray(mask, dtype=np.float32)
    image = np.ascontiguousarray(image, dtype=np.float32)
    in_maps = []
    for i in range(N_CORES):
        m = {
            "mask": mask[i * BPC : (i + 1) * BPC],
            "image": image[i * BPC : (i + 1) * BPC],
        }
        m.update(consts)
        in_maps.append(m)
    res = run_bass_kernel_spmd(nc, in_maps, list(range(N_CORES)), trace=trace, **kwargs)
    out = np.concatenate(
        [res.results[i]["out"].astype(np.float32) for i in range(N_CORES)], axis=0
    )
    # device stores planar [B, C, H, W]; reorder to [B, H, W, C] on host
    return np.ascontiguousarray(out.transpose(0, 2, 3, 1)), res


def kernel(mask: np.ndarray, image: np.ndarray) -> np.ndarray:
    out, _ = run(mask, image)
    return out.astype(np.float32)

